# revision 19
# baseline (speedup 1.0000x reference)
"""Trainium2 Bass kernel for the MultiHeadAttention transformer block.

Sharding: 8 cores, core c handles batch b=c//2 and query-row half
(c%2)*1024 .. +1024, all 8 heads.  Each core is fully independent
(no collectives).

v4 design:
  - QK matmuls run as row-tiled HEAD PAIRS (even head on PE rows 0:64,
    odd head on rows 64:128, concurrently).  Score granules are
    [128, 512] (one PSUM bank), double-buffered per consumer engine so
    the exp consumers never stall the PE.
  - softmax exp is split: Scalar engine (true Exp) takes the even
    head's granules, Vector engine takes the odd head's via the
    Schraudolph exp2 bit trick (bf16 bits = 32*t + 16256.5, t =
    4*log2(e)*logit straight from the matmul because Wq is pre-scaled
    by log2(e)/2 on the host).
  - masking is folded out of the exp: V rows are mask-zeroed on the
    projection copy, denominators come from col-tiled matmuls with
    the 0/1 mask column as stationary (4 accumulation groups at
    partitions 0/32/64/96 of one bank).
  - AV runs as col-tiled head pairs; denb/AV trail the exps by one
    key tile (software pipeline) so the in-order PE queue never parks
    behind an exp.
  - K/Q/V projections for the NEXT head pair are interleaved into the
    current pair's key loop; K is stored channel-tile-major like Q so
    each projection PSUM tile moves with a single copy.
  - K/Q copies ride the Scalar engine, V copies ride Vector (fused
    with the mask multiply), the LN elementwise tail is pipelined
    across both query halves.
"""

import sys

if "/opt/trn_rl_repo" not in sys.path:
    sys.path.insert(0, "/opt/trn_rl_repo")

import math

import numpy as np

import concourse.bacc as bacc
import concourse.bass as bass
import concourse.tile as tile
from concourse import mybir
from concourse.bass_utils import run_bass_kernel_spmd

H, D, DK, DV = 8, 512, 64, 64
B, L = 4, 2048
P = 128
LQ = L // 2          # query rows per core
NCORES = 8
EPS = 1e-5
F32 = mybir.dt.float32
BF16 = mybir.dt.bfloat16
I16 = mybir.dt.int16
F32R = mybir.dt.float32r
AF = mybir.ActivationFunctionType
Alu = mybir.AluOpType

DT = D // P     # 4 d-tiles
LKT = L // P    # 16 key tiles
NB = LQ // 512  # 2 psum-bank columns of queries

# host pre-scales Wq by log2(e)/2 so QK psum = 4*log2(e)*logit
QSCALE = math.log2(math.e) / 2.0
LN2_4 = math.log(2.0) / 4.0     # ACT: exp(psum * ln2/4) = e^logit
SCH_MUL = 32.0                  # DVE: bf16 bits = psum*32 + 16256.5
SCH_ADD = 16256.5

_CACHE = {}


def _bcast(ap, parts):
    """Partition-broadcast view of a [1, n] DRAM AP for DMA replication."""
    return ap.to_broadcast([parts] + list(ap.shape[1:]))


def _emit(nc, tc):
    qT = nc.dram_tensor("qT", [P, DT, LQ], BF16, kind="ExternalInput")
    kT = nc.dram_tensor("kT", [P, DT, L], BF16, kind="ExternalInput")
    vT = nc.dram_tensor("vT", [P, DT, L], BF16, kind="ExternalInput")
    qresT = nc.dram_tensor("qresT", [P, DT, LQ], BF16, kind="ExternalInput")
    WqT = nc.dram_tensor("WqT", [P, DT, D], BF16, kind="ExternalInput")
    WkT = nc.dram_tensor("WkT", [P, DT, D], BF16, kind="ExternalInput")
    WvT = nc.dram_tensor("WvT", [P, DT, D], BF16, kind="ExternalInput")
    fcwT = nc.dram_tensor("fcwT", [P, DT, D], BF16, kind="ExternalInput")
    mexpD = nc.dram_tensor("mexpD", [P, LKT, H * DV], BF16,
                           kind="ExternalInput")
    mcolD = nc.dram_tensor("mcolD", [P, LKT, H], BF16, kind="ExternalInput")
    vecs = nc.dram_tensor("vecs", [5, P, DT], F32, kind="ExternalInput")
    out = nc.dram_tensor("out", [P, DT, LQ], BF16, kind="ExternalOutput")

    with (
        tc.tile_pool(name="consts", bufs=1) as consts,
        tc.tile_pool(name="projout", bufs=1) as projout,
        tc.tile_pool(name="dramp", bufs=3, space="DRAM") as dramp,
        tc.tile_pool(name="ps", bufs=1, space="PSUM") as ps,
    ):
        # ---- constants resident for the whole kernel ----
        mcol_s = consts.tile([P, LKT, H], BF16)
        nc.sync.dma_start(out=mcol_s, in_=mcolD[:, :, :])
        gbT = consts.tile([P, 5, DT], F32)   # g0,b0,g1,b1,fcb as [p, dt]
        for i in range(5):
            nc.sync.dma_start(out=gbT[:, i, :], in_=vecs[i, :, :])
        eps_t = consts.tile([P, 1], F32)
        nc.vector.memset(eps_t, EPS)
        ones_st = consts.tile([P, 1], BF16)  # stats reduction column
        nc.vector.memset(ones_st, 1.0)
        ones_r1 = consts.tile([1, P], BF16)  # rank-1 broadcast row
        nc.vector.memset(ones_r1, 1.0)
        ones_r1f = consts.tile([1, P], F32)  # f32 variant for f32r moving
        nc.vector.memset(ones_r1f, 1.0)
        warm = consts.tile([P, 512], BF16)   # PE warm-up fodder
        nc.vector.memset(warm[:, :], 0.0)
        expw = consts.tile([1, 1], F32)      # preload the Exp table set
        nc.scalar.activation(out=expw, in_=eps_t[0:1, 0:1], func=AF.Exp,
                             bias=eps_t[0:1, :], scale=1.0)

        # ---- persistent big tiles ----
        QT_s = projout.tile([P, DT, LQ], BF16)        # Q^T  [ch, lq]
        KT_s = projout.tile([P, DT, L], BF16)         # K^T  [ch, lk]
        V_s = projout.tile([P, LKT, H * DV], BF16)    # V rows, mask-zeroed
        qresT_s = projout.tile([P, DT, LQ], BF16)

        with tc.tile_pool(name="statin", bufs=1) as statin:
            xbf = statin.tile([P, DT, LQ], BF16, tag="xbf")
            x2bf = statin.tile([P, DT, LQ], BF16, tag="x2bf")

            # ====== phases A+B fused: per-pair projections + attention ======
            with (
                tc.tile_pool(name="inp", bufs=1) as inp,
                tc.tile_pool(name="wts", bufs=1) as wts,
                tc.tile_pool(name="pT", bufs=16) as pTp,
                tc.tile_pool(name="mex", bufs=2) as mex,
                tc.tile_pool(name="rcp", bufs=3) as rcp,
                tc.tile_pool(name="bcsp", bufs=4) as bcsp,
            ):
                for w in range(12):
                    wps = ps.tile([P, 512], F32, tag="proj",
                                  name=f"warm{w}")
                    nc.tensor.matmul(wps[:, :], warm[:, 0:P], warm[:, :],
                                     start=True, stop=True)
                WqT_s = wts.tile([P, DT, D], BF16)
                qT_s = inp.tile([P, DT, LQ], BF16)
                WkT_s = wts.tile([P, DT, D], BF16)
                kT_s = inp.tile([P, DT, L], BF16)
                WvT_s = wts.tile([P, DT, D], BF16)
                vT_s = inp.tile([P, DT, L], BF16)
                for dt in range(DT):
                    nc.sync.dma_start(out=WkT_s[:, dt, :], in_=WkT[:, dt, :])
                    nc.sync.dma_start(out=kT_s[:, dt, 0:512],
                                      in_=kT[:, dt, 0:512])
                for dt in range(DT):
                    nc.sync.dma_start(out=WqT_s[:, dt, :], in_=WqT[:, dt, :])
                    nc.sync.dma_start(out=qT_s[:, dt, :], in_=qT[:, dt, :])
                for jb in range(1, 4):
                    for dt in range(DT):
                        nc.sync.dma_start(
                            out=kT_s[:, dt, jb * 512:(jb + 1) * 512],
                            in_=kT[:, dt, jb * 512:(jb + 1) * 512])
                for dt in range(DT):
                    nc.sync.dma_start(out=WvT_s[:, dt, :], in_=WvT[:, dt, :])
                    nc.sync.dma_start(out=vT_s[:, dt, :], in_=vT[:, dt, :])
                for dt in range(DT):
                    nc.sync.dma_start(out=qresT_s[:, dt, :],
                                      in_=qresT[:, dt, :])

                def emit_qproj(m, jb):
                    qps = ps.tile([P, 512], F32, tag="proj",
                                  name=f"psq{m}_{jb}")
                    for dt in range(DT):
                        nc.tensor.matmul(
                            qps[:, :],
                            WqT_s[:, dt, m * P:(m + 1) * P],
                            qT_s[:, dt, jb * 512:(jb + 1) * 512],
                            start=(dt == 0), stop=(dt == DT - 1))
                    nc.scalar.copy(
                        out=QT_s[:, m, jb * 512:(jb + 1) * 512],
                        in_=qps[:, :])

                def emit_kproj(m, jb):
                    kps = ps.tile([P, 512], F32, tag="proj",
                                  name=f"psk{m}_{jb}")
                    for dt in range(DT):
                        nc.tensor.matmul(
                            kps[:, :],
                            WkT_s[:, dt, m * P:(m + 1) * P],
                            kT_s[:, dt, jb * 512:(jb + 1) * 512],
                            start=(dt == 0), stop=(dt == DT - 1))
                    nc.scalar.copy(
                        out=KT_s[:, m, jb * 512:(jb + 1) * 512],
                        in_=kps[:, :])

                def emit_vproj(lk):
                    vps = ps.tile([P, 512], F32, tag="proj", name=f"psv{lk}")
                    for dt in range(DT):
                        nc.tensor.matmul(
                            vps[:, :],
                            vT_s[:, dt, lk * P:(lk + 1) * P],
                            WvT_s[:, dt, :],
                            start=(dt == 0), stop=(dt == DT - 1))
                    mx = mex.tile([P, H * DV], BF16, tag="mex")
                    nc.sync.dma_start(out=mx, in_=mexpD[:, lk, :])
                    # mask-zero the V rows while copying out of PSUM
                    nc.vector.tensor_mul(V_s[:, lk, :], vps[:, :], mx[:, :])

                def emit_attn_pair(mt, prefetch=None):
                    """prefetch: dict m -> [thunks] emitted inside the key
                    loop right after key tile m's QK matmuls."""
                    h0, h1 = 2 * mt, 2 * mt + 1
                    pf = prefetch or {}
                    avs = [ps.tile([P, 512], F32, tag="av", bufs=2,
                                   name=f"av{mt}_{j}") for j in range(NB)]
                    denb = ps.tile([P, 512], F32, tag="den", bufs=1,
                                   name=f"den{mt}")

                    def denb_av(m, pts):
                        st, sp = (m == 0), (m == LKT - 1)
                        for jb in range(NB):
                            r0, r1 = 64 * jb, 64 * jb + 32
                            nc.tensor.matmul(
                                denb[r0:r0 + 1, :], mcol_s[:, m, h0:h0 + 1],
                                pts[2 * jb][:, :], start=st, stop=sp,
                                tile_position=(0, r0))
                            nc.tensor.matmul(
                                denb[r1:r1 + 1, :], mcol_s[:, m, h1:h1 + 1],
                                pts[2 * jb + 1][:, :], start=st, stop=sp,
                                tile_position=(0, r1))
                        for jb in range(NB):
                            nc.tensor.matmul(
                                avs[jb][0:64, :],
                                V_s[:, m, h0 * DV:(h0 + 1) * DV],
                                pts[2 * jb][:, :], start=st, stop=sp,
                                tile_position=(0, 0))
                            nc.tensor.matmul(
                                avs[jb][64:128, :],
                                V_s[:, m, h1 * DV:(h1 + 1) * DV],
                                pts[2 * jb + 1][:, :], start=st, stop=sp,
                                tile_position=(0, 64))

                    prev = None
                    for m in range(LKT):
                        gs = []
                        msl = slice(m * P, (m + 1) * P)
                        for jb in range(NB):
                            qsl = slice(jb * 512, (jb + 1) * 512)
                            gA = ps.tile([P, 512], F32, tag="qkA", bufs=2,
                                         name=f"gA{mt}_{m}_{jb}")
                            gB = ps.tile([P, 512], F32, tag="qkB", bufs=2,
                                         name=f"gB{mt}_{m}_{jb}")
                            nc.tensor.matmul(
                                gA[:, :], KT_s[0:64, mt, msl],
                                QT_s[0:64, mt, qsl], start=True, stop=True)
                            nc.tensor.matmul(
                                gB[:, :], KT_s[64:128, mt, msl],
                                QT_s[64:128, mt, qsl], start=True, stop=True)
                            gs += [gA, gB]
                        for thunk in pf.get(m, ()):
                            thunk()
                        # software pipeline: denb/AV run one key tile behind
                        if prev is not None:
                            denb_av(*prev)
                        pts = []
                        for jb in range(NB):
                            ptA = pTp.tile([P, 512], BF16, tag="pt",
                                           name=f"ptA{mt}_{m}_{jb}")
                            ptB = pTp.tile([P, 512], BF16, tag="pt",
                                           name=f"ptB{mt}_{m}_{jb}")
                            nc.scalar.activation(out=ptA[:, :],
                                                 in_=gs[2 * jb][:, :],
                                                 func=AF.Exp, scale=LN2_4)
                            nc.vector.tensor_scalar(
                                out=ptB.bitcast(I16)[:, :],
                                in0=gs[2 * jb + 1][:, :],
                                scalar1=SCH_MUL, scalar2=SCH_ADD,
                                op0=Alu.mult, op1=Alu.add)
                            pts += [ptA, ptB]
                        prev = (m, pts)
                    denb_av(*prev)

                    # normalize: DRAM-bounce broadcast of denominators
                    for jb in range(NB):
                        rcd = dramp.tile([2, 512], F32, tag="rcd",
                                         name=f"rcd{mt}_{jb}")
                        r0, r1 = 64 * jb, 64 * jb + 32
                        dn0 = rcp.tile([1, 512], F32, tag="dn",
                                       name=f"dn0{mt}_{jb}")
                        dn1 = rcp.tile([1, 512], F32, tag="dn",
                                       name=f"dn1{mt}_{jb}")
                        nc.vector.tensor_copy(dn0[0:1, :], denb[r0:r0 + 1, :])
                        nc.vector.tensor_copy(dn1[0:1, :], denb[r1:r1 + 1, :])
                        nc.sync.dma_start(out=rcd[0:1, :], in_=dn0[0:1, :])
                        nc.sync.dma_start(out=rcd[1:2, :], in_=dn1[0:1, :])
                        bcs = bcsp.tile([P, 512], F32, tag="bcs")
                        nc.gpsimd.dma_start(out=bcs[0:64, :],
                                            in_=_bcast(rcd[0:1, :], 64))
                        nc.gpsimd.dma_start(out=bcs[64:128, :],
                                            in_=_bcast(rcd[1:2, :], 64))
                        nc.vector.reciprocal_approx_fast(out=bcs, in_=bcs)
                        nc.vector.tensor_mul(
                            xbf[:, mt, jb * 512:(jb + 1) * 512],
                            avs[jb][:, :], bcs[:, :])
                    # residual + stat inputs for this channel tile
                    if mt == DT - 1:
                        # last pair: keep the chain into LN0 stats on DVE
                        for jb in range(NB):
                            sl = slice(jb * 512, (jb + 1) * 512)
                            nc.vector.tensor_add(xbf[:, mt, sl],
                                                 xbf[:, mt, sl],
                                                 qresT_s[:, mt, sl])
                            nc.vector.tensor_mul(x2bf[:, mt, sl],
                                                 xbf[:, mt, sl],
                                                 xbf[:, mt, sl])
                    else:
                        nc.gpsimd.tensor_add(xbf[:, mt, :], xbf[:, mt, :],
                                             qresT_s[:, mt, :])
                        nc.gpsimd.tensor_mul(x2bf[:, mt, :], xbf[:, mt, :],
                                             xbf[:, mt, :])

                # head: only the projections pair 0's first key tiles need;
                # everything else is interleaved into the attention loops.
                emit_kproj(0, 0)
                for jb in range(NB):
                    emit_qproj(0, jb)

                def make_pf(mt):
                    """Prefetch schedule hosted by pair mt: its own
                    remaining kproj groups (group jb feeds key tiles
                    4jb..4jb+3) plus the NEXT pair's first kproj group and
                    both qproj groups."""
                    pf = {}

                    def add(m, th):
                        pf.setdefault(m, []).append(th)

                    for jb in (1, 2, 3):
                        add(4 * jb - 3, lambda j=jb: emit_kproj(mt, j))
                    if mt + 1 < DT:
                        add(9, lambda: emit_kproj(mt + 1, 0))
                        add(11, lambda: emit_qproj(mt + 1, 0))
                        add(13, lambda: emit_qproj(mt + 1, 1))
                    if mt == 0:
                        for m in range(LKT):
                            add(m, lambda lk=m: emit_vproj(lk))
                    return pf

                for mt in range(DT):
                    emit_attn_pair(mt, prefetch=make_pf(mt))

            # ============ phase C: LN0 -> fc -> LN1 (all ^T, bf16) ========
            with (
                tc.tile_pool(name="lnp", bufs=1) as lnp,
                tc.tile_pool(name="chain", bufs=2) as chain,
                tc.tile_pool(name="bcB", bufs=4) as bcB,
                tc.tile_pool(name="wfc", bufs=1) as wfc,
            ):
                outT = lnp.tile([P, DT, LQ], BF16)
                y2bf = lnp.tile([P, DT, LQ], BF16)

                def ln_stats(xb, x2b, label):
                    """Col-tiled 4-up stats: rows 0/32 = s1/s2 of half 0,
                    rows 64/96 = s1/s2 of half 1."""
                    sps = ps.tile([P, 512], F32, tag="den", bufs=1,
                                  name=f"st{label}")
                    for kt in range(DT):
                        st, sp = (kt == 0), (kt == DT - 1)
                        for nh in range(NB):
                            sl = slice(nh * 512, (nh + 1) * 512)
                            r = 64 * nh
                            nc.tensor.matmul(sps[r:r + 1, :], ones_st[:, :],
                                             xb[:, kt, sl], start=st, stop=sp,
                                             tile_position=(0, r))
                            nc.tensor.matmul(sps[r + 32:r + 33, :],
                                             ones_st[:, :],
                                             x2b[:, kt, sl], start=st, stop=sp,
                                             tile_position=(0, r + 32))
                    return sps

                def ln_half(sps, xb, g_idx, b_idx, nh, label, final_out=None):
                    """One 512-query half of a transposed LayerNorm, applied
                    in place on the bf16 tile xb."""
                    sl = slice(nh * 512, (nh + 1) * 512)
                    r = 64 * nh
                    s1 = sps[r:r + 1, :]
                    s2 = sps[r + 32:r + 33, :]
                    mu = chain.tile([1, 512], F32, tag="mu")
                    nc.vector.tensor_scalar_mul(mu, s1, 1.0 / D)
                    var = chain.tile([1, 512], F32, tag="var")
                    nc.vector.tensor_mul(var, mu, mu)
                    msq = chain.tile([1, 512], F32, tag="msq")
                    nc.vector.tensor_scalar_mul(msq, s2, 1.0 / D)
                    nc.vector.tensor_sub(var, msq, var)
                    nc.scalar.activation(out=var, in_=var, func=AF.Sqrt,
                                         bias=eps_t[0:1, :])
                    rstd = chain.tile([1, 512], F32, tag="rstd")
                    nc.vector.reciprocal_approx_fast(out=rstd, in_=var)
                    mrb = chain.tile([1, 2, 512], BF16, tag="mrb")
                    nc.vector.tensor_copy(mrb[:, 0, :], mu[0:1, :])
                    nc.vector.tensor_copy(mrb[:, 1, :], rstd[0:1, :])
                    mu_b = ps.tile([P, 512], F32, tag="av", bufs=2,
                                   name=f"mb{label}{nh}")
                    nc.tensor.matmul(mu_b[:, :], ones_r1[:, :], mrb[:, 0, :],
                                     start=True, stop=True)
                    rstd_b = ps.tile([P, 512], F32, tag="av", bufs=2,
                                     name=f"rb{label}{nh}")
                    nc.tensor.matmul(rstd_b[:, :], ones_r1[:, :], mrb[:, 1, :],
                                     start=True, stop=True)
                    for kt in range(DT):
                        # free-varying normalize on DVE, reading the PSUM
                        # broadcasts directly (no SBUF bounce copies)
                        nc.vector.tensor_sub(xb[:, kt, sl], xb[:, kt, sl],
                                             mu_b[:, :])
                        nc.vector.tensor_mul(xb[:, kt, sl], xb[:, kt, sl],
                                             rstd_b[:, :])
                        # gamma/beta are per-partition scalars: ACT applies
                        tgt = final_out if final_out is not None else xb
                        nc.scalar.activation(
                            out=tgt[:, kt, sl], in_=xb[:, kt, sl],
                            func=AF.Identity, bias=gbT[:, b_idx, kt:kt + 1],
                            scale=gbT[:, g_idx, kt:kt + 1])

                def emit_fc(nh):
                    sl = slice(nh * 512, (nh + 1) * 512)
                    for m in range(DT):
                        fps = ps.tile([P, 512], F32, tag="av", bufs=2,
                                      name=f"fc{m}_{nh}")
                        for dt in range(DT):
                            nc.tensor.matmul(
                                fps[:, :],
                                fcwT_s[:, dt, m * P:(m + 1) * P],
                                xbf[:, dt, sl],
                                start=(dt == 0), stop=(dt == DT - 1))
                        nc.scalar.activation(
                            out=ybf[:, m, sl], in_=fps[:, :],
                            func=AF.Identity, bias=gbT[:, 4, m:m + 1])
                        nc.gpsimd.tensor_add(ybf[:, m, sl], ybf[:, m, sl],
                                             xbf[:, m, sl])
                        nc.gpsimd.tensor_mul(y2bf[:, m, sl], ybf[:, m, sl],
                                             ybf[:, m, sl])

                fcwT_s = wfc.tile([P, DT, D], BF16)
                nc.sync.dma_start(out=fcwT_s, in_=fcwT[:, :, :])
                ybf = statin.tile([P, DT, LQ], BF16, tag="x2bf")

                # LN0 both halves (in place on xbf = LN0 output, bf16),
                # pipelined: fc of half 0 overlaps LN0 of half 1
                spsA = ln_stats(xbf, x2bf, "a")
                ln_half(spsA, xbf, 0, 1, 0, "a")
                ln_half(spsA, xbf, 0, 1, 1, "a")
                emit_fc(0)
                emit_fc(1)
                spsB = ln_stats(ybf, y2bf, "b")
                for nh in range(NB):
                    sl = slice(nh * 512, (nh + 1) * 512)
                    ln_half(spsB, ybf, 2, 3, nh, "b", final_out=outT)
                    for kt in range(DT):
                        nc.sync.dma_start(out=out[:, kt, sl],
                                          in_=outT[:, kt, sl])


def _build():
    if "nc" in _CACHE:
        return _CACHE["nc"]
    nc = bacc.Bacc(None, target_bir_lowering=False, debug=False)
    with tile.TileContext(nc) as tc:
        _emit(nc, tc)
    nc.compile()
    _CACHE["nc"] = nc
    return nc


def _prep_in_maps(q, k, v, mask, Wq, Wk, Wv, fc_w, fc_b, g0, b0, g1, b1):
    q = np.asarray(q, np.float32)
    k = np.asarray(k, np.float32)
    v = np.asarray(v, np.float32)
    mask = np.asarray(mask)
    bf = mybir.dt.np(BF16)

    def ptile(a):
        # [n, m] -> transpose -> [m(=tiles*128), n] -> [128, tiles, n]
        t = np.asarray(a, np.float32).T
        return np.ascontiguousarray(
            t.reshape(DT, P, t.shape[1]).transpose(1, 0, 2))

    WqTh = ptile(np.asarray(Wq, np.float32) * QSCALE).astype(bf)
    WkTh = ptile(Wk).astype(bf)
    WvTh = ptile(Wv).astype(bf)
    fcwTh = ptile(fc_w).astype(bf)
    vecs = np.stack([np.asarray(x, np.float32).reshape(DT, P).T
                     for x in (g0, b0, g1, b1, fc_b)])
    vecs = np.ascontiguousarray(vecs)

    in_maps = []
    for c in range(NCORES):
        b = c // 2
        r0 = (c % 2) * LQ
        qTb = ptile(q[b][r0:r0 + LQ]).astype(bf)
        kTb = ptile(k[b]).astype(bf)
        vTb = ptile(v[b]).astype(bf)
        qrTb = ptile(q[b][r0:r0 + LQ]).astype(bf)
        mh = np.zeros((P, LKT, H), np.float32)
        for h in range(H):
            mh[:, :, h] = mask[h * B + b].reshape(LKT, P).T
        mexp = np.broadcast_to(mh[:, :, :, None],
                               (P, LKT, H, DV)).reshape(P, LKT, H * DV)
        in_maps.append({
            "qT": qTb, "kT": kTb, "vT": vTb, "qresT": qrTb,
            "WqT": WqTh, "WkT": WkTh, "WvT": WvTh, "fcwT": fcwTh,
            "mexpD": np.ascontiguousarray(mexp).astype(bf),
            "mcolD": np.ascontiguousarray(mh).astype(bf),
            "vecs": vecs,
        })
    return in_maps


def kernel(q, k, v, mask, Wq, Wk, Wv, fc_w, fc_b, g0, b0, g1, b1):
    in_maps = _prep_in_maps(q, k, v, mask, Wq, Wk, Wv, fc_w, fc_b,
                            g0, b0, g1, b1)
    nc = _build()
    res = run_bass_kernel_spmd(nc, in_maps, core_ids=list(range(NCORES)))
    outf = np.empty((B, L, D), np.float32)
    for c in range(NCORES):
        b = c // 2
        r0 = (c % 2) * LQ
        o = np.asarray(res.results[c]["out"], np.float32)  # [128, DT, LQ]
        outf[b, r0:r0 + LQ, :] = o.transpose(2, 1, 0).reshape(LQ, D)
    return outf
